# revision 16
# baseline (speedup 1.0000x reference)
"""Trainium2 Bass kernel for CrossEncoderMTL: weighted layer pooling + masked
mean + section-routed adapter + reg/ord heads, data-parallel over batch B
across 8 NeuronCores.

Design (per core, 32 samples):
- The 1 GiB hidden_states stream is the roofline; it is consumed by the
  VECTOR engine (fp32-exact, 1 elem/lane/cyc) doing the masked (l, s)
  accumulation with per-partition scalars, so the tensor engine never streams
  the big fp32 tensor at its 4-cycles/column fp32 rate.
- The remaining 128-partition reduction runs on the PE with 0/1 routing
  weights, accumulating feats for groups of 8 samples directly in PSUM.
- The section-routed adapter gathers per-sample expert weights on the host
  (numpy), and the up-projection is folded into the heads algebraically:
  out = feats @ [W_reg|W_ord] + gelu(feats @ Wd + bd) @ (Wu @ [W_reg|W_ord])
      + (bu @ [W_reg|W_ord] + [b_reg|b_ord]),
  since only reg/ord are returned. Per-sample matmuls are batched per group
  of 8 with a diagonal-extraction (mask + grouped reduce) on the DVE.
- The adapter + heads run per group of 8 samples, pipelined inside the DMA
  stream; big constants are staggered into the sync HWDGE FIFO so they never
  compete with the stream at startup.

Everything on device is fp32 (PSUM accumulation in fp32); measured output is
bit-close to the jax fp32 reference (max rel err ~4e-7).
"""

import numpy as np

L, B, S, H, K = 4, 256, 256, 1024, 64
NCORES = 8
BL = B // NCORES          # 32 samples per core
NH = H // 128             # 8 h-chunks (h = 8*p + r mapping)
NG = 4                    # sample groups per core
GS = BL // NG             # 8 samples per group

_CACHE = {}
LAST_RESULT = None        # BassKernelResults of the most recent run


def _build_module(act_fn="Gelu"):
    from contextlib import ExitStack
    from concourse import bacc, mybir, tile

    f32 = mybir.dt.float32
    AT = mybir.AluOpType
    nc = bacc.Bacc(
        "TRN2", target_bir_lowering=False, debug=False, num_devices=NCORES
    )

    hs = nc.dram_tensor("hs", [L, BL, S, H], f32, kind="ExternalInput")
    # cs[p, (l*2+c)*BL + b] = softmax(w)[l] * mask[b, c*128+p] / msum[b]
    cs = nc.dram_tensor("cs", [128, L * 2 * BL], f32, kind="ExternalInput")
    # ones8[p, j*GS + i] = 1.0 iff i == j  (routes sample b to psum row b%GS)
    ones8 = nc.dram_tensor("ones8", [128, GS * GS], f32, kind="ExternalInput")
    # wd2[p, ((g*NH + r)*GS + bb)*K + k] = W_down[sec[g*GS+bb], 8p+r, k]
    wd2 = nc.dram_tensor("wd2", [128, NG * NH * GS * K], f32, kind="ExternalInput")
    # wro[p, r*5+j] = [W_reg | W_ord][8p+r, j]
    wro = nc.dram_tensor("wro", [128, NH * 5], f32, kind="ExternalInput")
    # wuro[k, b*5+j] = (W_up[sec[b]] @ [W_reg|W_ord])[k, j]
    wuro = nc.dram_tensor("wuro", [K, BL * 5], f32, kind="ExternalInput")
    m64 = nc.dram_tensor("m64", [GS, GS * K], f32, kind="ExternalInput")
    m5 = nc.dram_tensor("m5", [GS, GS * 5], f32, kind="ExternalInput")
    bd8 = nc.dram_tensor("bd8", [GS, NG * K], f32, kind="ExternalInput")
    bro8 = nc.dram_tensor("bro8", [GS, NG * 5], f32, kind="ExternalInput")
    ident = nc.dram_tensor("ident", [GS, GS], f32, kind="ExternalInput")
    out = nc.dram_tensor("out", [BL, 5], f32, kind="ExternalOutput")

    with tile.TileContext(nc) as tc:
        with ExitStack() as ctx:
            consts = ctx.enter_context(tc.tile_pool(name="consts", bufs=1))
            hs_pool = ctx.enter_context(tc.tile_pool(name="hsp", bufs=6))
            apool = ctx.enter_context(tc.tile_pool(name="acc", bufs=2))
            work = ctx.enter_context(tc.tile_pool(name="work", bufs=2))
            pt_pool = ctx.enter_context(tc.tile_pool(name="pt", bufs=2, space="PSUM"))
            pg_pool = ctx.enter_context(tc.tile_pool(name="pg", bufs=1, space="PSUM"))
            pf_pool = ctx.enter_context(tc.tile_pool(name="pfg", bufs=1, space="PSUM"))

            # Small constants load on the scalar-engine HWDGE ring (doesn't
            # head-of-line-block the hidden_states stream on the sync ring).
            def cload(dram, shape):
                t = consts.tile(shape, f32, tag=dram.name)
                nc.scalar.dma_start(t[:], dram.ap())
                return t

            id_sb = cload(ident, [GS, GS])
            ones_sb = cload(ones8, [128, GS * GS])
            cs_sb = cload(cs, [128, L * 2 * BL])
            wro_sb = cload(wro, [128, NH * 5])
            wuro_sb = cload(wuro, [K, BL * 5])
            m64_sb = cload(m64, [GS, GS * K])
            m5_sb = cload(m5, [GS, GS * 5])
            bd8_sb = cload(bd8, [GS, NG * K])
            bro8_sb = cload(bro8, [GS, NG * 5])
            # wd2 (8 MiB) streams in per-group on the sync ring (see loop)
            wd_sb = consts.tile([128, NG * NH * GS * K], f32, tag="wd2")
            featsT = consts.tile([128, NH * BL], f32, tag="featsT")
            h1T = consts.tile([K, BL], f32, tag="h1T")

            from concourse import mybir as _mb
            GCOLS = NH * GS * K  # columns per wd2 group chunk

            def warmup(n):
                with tc.tile_pool(name="pwarm", bufs=1, space="PSUM") as pwarm:
                    wps = pwarm.tile([GS, GS], f32)
                    for _ in range(n):
                        nc.tensor.matmul(wps[:], id_sb[:], id_sb[:],
                                         start=True, stop=True)

            warmup(64)

            hs_ap = hs.ap()
            for b in range(BL):
                g, m = divmod(b, GS)
                if b in (2, 10, 18, 26):
                    # stagger the expert-weight chunks into the stream, one
                    # group (~100us) ahead of their use
                    gg = (b - 2) // GS
                    blk = slice(gg * GCOLS, (gg + 1) * GCOLS)
                    nc.sync.dma_start(wd_sb[:, blk], wd2.ap()[:, blk])
                if b == BL - 1:
                    warmup(48)  # re-warm HAM for the tail chain
                # ---- DVE masked accumulation over (l, c) for sample b ----
                acc = None
                flip = False
                for l in range(L):
                    t = hs_pool.tile([128, 2 * 1024], f32, tag="hst")
                    src = hs_ap[l, b].rearrange("(c p) h -> p c h", p=128)
                    nc.sync.dma_start(t[:].rearrange("p (c h) -> p c h", c=2), src)
                    for c in range(2):
                        sidx = (l * 2 + c) * BL + b
                        sc = cs_sb[:, sidx:sidx + 1]
                        blk = t[:, c * 1024:(c + 1) * 1024]
                        if acc is None:
                            acc = apool.tile([128, 1024], f32, tag="accA")
                            nc.vector.tensor_scalar_mul(acc[:], blk, sc)
                        else:
                            nxt = apool.tile([128, 1024], f32,
                                             tag="accB" if flip else "accA")
                            nc.vector.scalar_tensor_tensor(
                                nxt[:], blk, sc, acc[:], AT.mult, AT.add)
                            acc = nxt
                        flip = not flip
                # ---- PE partition-reduction: feats row b%GS of group g ----
                if m == 0:
                    pfg = pf_pool.tile([GS, H], f32, tag="pfg")
                lhsE = ones_sb[:, m * GS:(m + 1) * GS]
                for hh in range(2):
                    nc.tensor.matmul(
                        pfg[:, hh * 512:(hh + 1) * 512],
                        lhsE, acc[:, hh * 512:(hh + 1) * 512],
                        start=(m == 0), stop=(m == GS - 1),
                    )
                if m != GS - 1:
                    continue
                # ---- group complete: adapter + heads for samples g*8..g*8+7 ----
                fg = work.tile([GS, H], f32, tag="fg")
                nc.vector.tensor_copy(fg[:], pfg[:])
                fview = fg[:].rearrange("p (q r) -> p r q", r=NH)
                for r in range(NH):
                    pt = pt_pool.tile([128, GS], f32, tag="pt")
                    nc.tensor.transpose(pt[:], fview[:, r], id_sb[:])
                    nc.vector.tensor_copy(
                        featsT[:, r * BL + g * GS:r * BL + (g + 1) * GS], pt[:])
                # batched down-projection for the group; diagonal wanted
                bigD = pg_pool.tile([GS, GS * K], f32, tag="bigD")
                for r in range(NH):
                    nc.tensor.matmul(
                        bigD[:],
                        featsT[:, r * BL + g * GS:r * BL + (g + 1) * GS],
                        wd_sb[:, (g * NH + r) * GS * K:(g * NH + r + 1) * GS * K],
                        start=(r == 0), stop=(r == NH - 1),
                    )
                zm = work.tile([GS, GS * K], f32, tag="zm")
                nc.vector.tensor_mul(zm[:], bigD[:], m64_sb[:])
                zg = work.tile([GS, K], f32, tag="zg")
                nc.vector.tensor_reduce(
                    zg[:], zm[:].rearrange("p (g j) -> p j g", j=K),
                    _mb.AxisListType.X, AT.add)
                zb = work.tile([GS, K], f32, tag="zb")
                nc.vector.tensor_add(zb[:], zg[:], bd8_sb[:, g * K:(g + 1) * K])
                h1g = work.tile([GS, K], f32, tag="h1g")
                nc.scalar.activation(
                    h1g[:], zb[:], getattr(_mb.ActivationFunctionType, act_fn))
                pth = pt_pool.tile([K, GS], f32, tag="pt")
                nc.tensor.transpose(pth[:], h1g[:], id_sb[:])
                nc.vector.tensor_copy(h1T[:, g * GS:(g + 1) * GS], pth[:])
                # heads: feats part + adapter part (diagonal-extracted)
                pB = pg_pool.tile([GS, 5], f32, tag="pB")
                for r in range(NH):
                    nc.tensor.matmul(
                        pB[:],
                        featsT[:, r * BL + g * GS:r * BL + (g + 1) * GS],
                        wro_sb[:, r * 5:(r + 1) * 5],
                        start=(r == 0), stop=(r == NH - 1),
                    )
                pA = pg_pool.tile([GS, GS * 5], f32, tag="pA")
                nc.tensor.matmul(
                    pA[:], h1T[:, g * GS:(g + 1) * GS],
                    wuro_sb[:, g * GS * 5:(g + 1) * GS * 5],
                    start=True, stop=True)
                am = work.tile([GS, GS * 5], f32, tag="am")
                nc.vector.tensor_mul(am[:], pA[:], m5_sb[:])
                rg = work.tile([GS, 5], f32, tag="rg")
                nc.vector.tensor_reduce(
                    rg[:], am[:].rearrange("p (g j) -> p j g", j=5),
                    _mb.AxisListType.X, AT.add)
                o1 = work.tile([GS, 5], f32, tag="o1")
                nc.vector.tensor_add(o1[:], pB[:], rg[:])
                og = work.tile([GS, 5], f32, tag="og")
                nc.vector.tensor_add(og[:], o1[:], bro8_sb[:, g * 5:(g + 1) * 5])
                nc.sync.dma_start(out.ap()[g * GS:(g + 1) * GS, :], og[:])

    nc.compile()
    return nc


def _softmax(x):
    e = np.exp(x - x.max())
    return e / e.sum()


def _prepare_inputs(hidden_states, attention_mask, section_id, layer_weights,
                    W_down, b_down, W_up, b_up, W_reg, b_reg, W_ord, b_ord):
    hidden_states = np.asarray(hidden_states, dtype=np.float32)
    mask = np.asarray(attention_mask)
    sec = np.asarray(section_id).astype(np.int64)
    lw = np.asarray(layer_weights, dtype=np.float32)
    W_down = np.asarray(W_down, dtype=np.float32)
    b_down = np.asarray(b_down, dtype=np.float32)
    W_up = np.asarray(W_up, dtype=np.float32)
    b_up = np.asarray(b_up, dtype=np.float32)
    W_reg = np.asarray(W_reg, dtype=np.float32)
    b_reg = np.asarray(b_reg, dtype=np.float32)
    W_ord = np.asarray(W_ord, dtype=np.float32)
    b_ord = np.asarray(b_ord, dtype=np.float32)

    w = _softmax(lw)
    mf = mask.astype(np.float32)
    msum = np.maximum(mf.sum(axis=1), 1e-6)
    cmask = mf / msum[:, None]
    base = cmask.reshape(B, 2, 128).transpose(2, 1, 0)          # [p, c, b]
    coeff_all = w[None, :, None, None] * base[:, None, :, :]    # [p, l, c, b]

    wro_np = np.concatenate([W_reg, W_ord], axis=1)             # [H, 5]
    wro_dev = np.ascontiguousarray(wro_np.reshape(128, NH * 5))
    wd_all = W_down[sec]                                        # [B, H, K]
    wd_pbrk = wd_all.reshape(B, 128, NH, K).transpose(1, 0, 2, 3)  # [p, b, r, k]
    wu_ro = np.einsum("ekh,hj->ekj", W_up, wro_np)              # [NSEC, K, 5]
    wuro_all = wu_ro[sec].transpose(1, 0, 2)                    # [K, B, 5]
    bd_all = b_down[sec]                                        # [B, K]
    bro_all = b_up[sec] @ wro_np + np.concatenate([b_reg, b_ord])[None]

    m64_np = np.zeros((GS, GS * K), np.float32)
    m5_np = np.zeros((GS, GS * 5), np.float32)
    for i in range(GS):
        m64_np[i, i * K:(i + 1) * K] = 1.0
        m5_np[i, i * 5:(i + 1) * 5] = 1.0
    ones_np = np.zeros((128, GS * GS), np.float32)
    for j in range(GS):
        ones_np[:, j * GS + j] = 1.0
    ident = np.eye(GS, dtype=np.float32)

    in_maps = []
    for core in range(NCORES):
        sl = slice(core * BL, (core + 1) * BL)
        wd_core = wd_pbrk[:, sl]                                # [p, BL, r, k]
        wd2_np = wd_core.reshape(128, NG, GS, NH, K).transpose(0, 1, 3, 2, 4)
        in_maps.append({
            "hs": np.ascontiguousarray(hidden_states[:, sl]),
            "cs": np.ascontiguousarray(
                coeff_all[:, :, :, sl].reshape(128, L * 2 * BL)),
            "ones8": ones_np,
            "wd2": np.ascontiguousarray(wd2_np.reshape(128, NG * NH * GS * K)),
            "wro": wro_dev,
            "wuro": np.ascontiguousarray(wuro_all[:, sl].reshape(K, BL * 5)),
            "m64": m64_np,
            "m5": m5_np,
            "bd8": np.ascontiguousarray(
                bd_all[sl].reshape(NG, GS, K).transpose(1, 0, 2)
                .reshape(GS, NG * K)),
            "bro8": np.ascontiguousarray(
                bro_all[sl].reshape(NG, GS, 5).transpose(1, 0, 2)
                .reshape(GS, NG * 5)),
            "ident": ident,
        })
    return in_maps


def get_module(act_fn="Gelu"):
    key = "nc_" + act_fn
    if key not in _CACHE:
        _CACHE[key] = _build_module(act_fn)
    return _CACHE[key]


def kernel(hidden_states, attention_mask, section_id, layer_weights,
           W_down, b_down, W_up, b_up, W_reg, b_reg, W_ord, b_ord):
    global LAST_RESULT
    from concourse.bass_utils import run_bass_kernel_spmd

    in_maps = _prepare_inputs(
        hidden_states, attention_mask, section_id, layer_weights,
        W_down, b_down, W_up, b_up, W_reg, b_reg, W_ord, b_ord)
    nc = get_module()
    res = run_bass_kernel_spmd(nc, in_maps, list(range(NCORES)))
    LAST_RESULT = res
    out = np.concatenate([res.results[c]["out"] for c in range(NCORES)], axis=0)
    reg = np.ascontiguousarray(out[:, 0])
    ord_logits = np.ascontiguousarray(out[:, 1:5])
    return reg, ord_logits


# revision 18
# speedup vs baseline: 1.1559x; 1.1559x over previous
"""Trainium2 Bass kernel for CrossEncoderMTL: weighted layer pooling + masked
mean + section-routed adapter + reg/ord heads, data-parallel over batch B
across 8 NeuronCores.

Design (per core, 32 samples):
- The 1 GiB hidden_states stream is the roofline; it is consumed by the
  VECTOR engine (fp32-exact, 1 elem/lane/cyc) doing the masked (l, s)
  accumulation with per-partition scalars, so the tensor engine never streams
  the big fp32 tensor at its 4-cycles/column fp32 rate.
- The remaining 128-partition reduction runs on the PE with 0/1 routing
  weights, accumulating feats for groups of 8 samples directly in PSUM.
- The section-routed adapter gathers per-sample expert weights on the host
  (numpy), and the up-projection is folded into the heads algebraically:
  out = feats @ [W_reg|W_ord] + gelu(feats @ Wd + bd) @ (Wu @ [W_reg|W_ord])
      + (bu @ [W_reg|W_ord] + [b_reg|b_ord]),
  since only reg/ord are returned. Per-sample matmuls are batched per group
  of 8 with a diagonal-extraction (mask + grouped reduce) on the DVE.
- The adapter + heads run per group of 8 samples, pipelined inside the DMA
  stream; big constants are staggered into the sync HWDGE FIFO so they never
  compete with the stream at startup.

Everything on device is fp32 (PSUM accumulation in fp32); measured output is
bit-close to the jax fp32 reference (max rel err ~4e-7).
"""

import numpy as np

L, B, S, H, K = 4, 256, 256, 1024, 64
NCORES = 8
BL = B // NCORES          # 32 samples per core
NH = H // 128             # 8 h-chunks (h = 8*p + r mapping)
NG = 4                    # sample groups per core
GS = BL // NG             # 8 samples per group

_CACHE = {}
LAST_RESULT = None        # BassKernelResults of the most recent run


def _build_module(act_fn="Gelu"):
    from contextlib import ExitStack
    from concourse import bacc, mybir, tile

    f32 = mybir.dt.float32
    AT = mybir.AluOpType
    nc = bacc.Bacc(
        "TRN2", target_bir_lowering=False, debug=False, num_devices=NCORES
    )

    hs = nc.dram_tensor("hs", [L, BL, S, H], f32, kind="ExternalInput")
    # cs[p, (l*2+c)*BL + b] = softmax(w)[l] * mask[b, c*128+p] / msum[b]
    cs = nc.dram_tensor("cs", [128, L * 2 * BL], f32, kind="ExternalInput")
    # ones8[p, j*GS + i] = 1.0 iff i == j  (routes sample b to psum row b%GS)
    ones8 = nc.dram_tensor("ones8", [128, GS * GS], f32, kind="ExternalInput")
    # wd2[p, ((g*NH + r)*GS + bb)*K + k] = W_down[sec[g*GS+bb], 8p+r, k]
    wd2 = nc.dram_tensor("wd2", [128, NG * NH * GS * K], f32, kind="ExternalInput")
    # wro[p, r*5+j] = [W_reg | W_ord][8p+r, j]
    wro = nc.dram_tensor("wro", [128, NH * 5], f32, kind="ExternalInput")
    # wuro[k, b*5+j] = (W_up[sec[b]] @ [W_reg|W_ord])[k, j]
    wuro = nc.dram_tensor("wuro", [K, BL * 5], f32, kind="ExternalInput")
    m64 = nc.dram_tensor("m64", [GS, GS * K], f32, kind="ExternalInput")
    m5 = nc.dram_tensor("m5", [GS, GS * 5], f32, kind="ExternalInput")
    bd8 = nc.dram_tensor("bd8", [GS, NG * K], f32, kind="ExternalInput")
    bro8 = nc.dram_tensor("bro8", [GS, NG * 5], f32, kind="ExternalInput")
    ident = nc.dram_tensor("ident", [GS, GS], f32, kind="ExternalInput")
    out = nc.dram_tensor("out", [BL, 5], f32, kind="ExternalOutput")

    with tile.TileContext(nc) as tc:
        with ExitStack() as ctx:
            consts = ctx.enter_context(tc.tile_pool(name="consts", bufs=1))
            hs_pool = ctx.enter_context(tc.tile_pool(name="hsp", bufs=6))
            apool = ctx.enter_context(tc.tile_pool(name="acc", bufs=2))
            work = ctx.enter_context(tc.tile_pool(name="work", bufs=2))
            pt_pool = ctx.enter_context(tc.tile_pool(name="pt", bufs=2, space="PSUM"))
            pg_pool = ctx.enter_context(tc.tile_pool(name="pg", bufs=1, space="PSUM"))
            pf_pool = ctx.enter_context(tc.tile_pool(name="pfg", bufs=1, space="PSUM"))

            # Small constants load on the scalar-engine HWDGE ring (doesn't
            # head-of-line-block the hidden_states stream on the sync ring).
            def cload(dram, shape):
                t = consts.tile(shape, f32, tag=dram.name)
                nc.scalar.dma_start(t[:], dram.ap())
                return t

            id_sb = cload(ident, [GS, GS])
            ones_sb = cload(ones8, [128, GS * GS])
            cs_sb = cload(cs, [128, L * 2 * BL])
            wro_sb = cload(wro, [128, NH * 5])
            wuro_sb = cload(wuro, [K, BL * 5])
            m64_sb = cload(m64, [GS, GS * K])
            m5_sb = cload(m5, [GS, GS * 5])
            bd8_sb = cload(bd8, [GS, NG * K])
            bro8_sb = cload(bro8, [GS, NG * 5])
            # wd2 (8 MiB) streams in per-group on the sync ring (see loop)
            wd_sb = consts.tile([128, NG * NH * GS * K], f32, tag="wd2")
            featsT = consts.tile([128, NH * BL], f32, tag="featsT")
            h1T = consts.tile([K, BL], f32, tag="h1T")

            from concourse import mybir as _mb
            GCOLS = NH * GS * K  # columns per wd2 group chunk

            def warmup(n):
                with tc.tile_pool(name="pwarm", bufs=1, space="PSUM") as pwarm:
                    wps = pwarm.tile([GS, GS], f32)
                    for _ in range(n):
                        nc.tensor.matmul(wps[:], id_sb[:], id_sb[:],
                                         start=True, stop=True)

            warmup(64)

            hs_ap = hs.ap()
            for b in range(BL):
                g, m = divmod(b, GS)
                if b in (2, 10, 18, 26):
                    # stagger the expert-weight chunks into the stream, one
                    # group (~100us) ahead of their use
                    gg = (b - 2) // GS
                    blk = slice(gg * GCOLS, (gg + 1) * GCOLS)
                    nc.sync.dma_start(wd_sb[:, blk], wd2.ap()[:, blk])
                # ---- DVE masked accumulation over (l, c) for sample b ----
                acc = None
                flip = False
                for l in range(L):
                    t = hs_pool.tile([128, 2 * 1024], f32, tag="hst")
                    src = hs_ap[l, b].rearrange("(c p) h -> p c h", p=128)
                    nc.sync.dma_start(t[:].rearrange("p (c h) -> p c h", c=2), src)
                    for c in range(2):
                        sidx = (l * 2 + c) * BL + b
                        sc = cs_sb[:, sidx:sidx + 1]
                        blk = t[:, c * 1024:(c + 1) * 1024]
                        if acc is None:
                            acc = apool.tile([128, 1024], f32, tag="accA")
                            nc.vector.tensor_scalar_mul(acc[:], blk, sc)
                            if b == BL - 1:
                                warm_src = acc
                        else:
                            nxt = apool.tile([128, 1024], f32,
                                             tag="accB" if flip else "accA")
                            nc.vector.scalar_tensor_tensor(
                                nxt[:], blk, sc, acc[:], AT.mult, AT.add)
                            acc = nxt
                        flip = not flip
                if b == BL - 1:
                    # Dependency-timed HAM warmup: these dummy matmuls read
                    # sample 31's first acc tile, so the PE runs them during
                    # the final DVE chain -- warm right as the tail starts.
                    with tc.tile_pool(name="pwtail", bufs=1,
                                      space="PSUM") as pwt:
                        wps2 = pwt.tile([GS, 512], f32)
                        for _ in range(8):
                            nc.tensor.matmul(
                                wps2[:], id_sb[:], warm_src[0:GS, 0:512],
                                start=True, stop=True)
                # ---- PE partition-reduction: feats row b%GS of group g ----
                if m == 0:
                    pfg = pf_pool.tile([GS, H], f32, tag="pfg")
                lhsE = ones_sb[:, m * GS:(m + 1) * GS]
                for hh in range(2):
                    nc.tensor.matmul(
                        pfg[:, hh * 512:(hh + 1) * 512],
                        lhsE, acc[:, hh * 512:(hh + 1) * 512],
                        start=(m == 0), stop=(m == GS - 1),
                    )
                if m != GS - 1:
                    continue
                # ---- group complete: adapter + heads for samples g*8..g*8+7 ----
                fg = work.tile([GS, H], f32, tag="fg")
                nc.vector.tensor_copy(fg[:], pfg[:])
                fview = fg[:].rearrange("p (q r) -> p r q", r=NH)
                for r in range(NH):
                    pt = pt_pool.tile([128, GS], f32, tag="pt")
                    nc.tensor.transpose(pt[:], fview[:, r], id_sb[:])
                    nc.vector.tensor_copy(
                        featsT[:, r * BL + g * GS:r * BL + (g + 1) * GS], pt[:])
                # batched down-projection for the group; diagonal wanted
                bigD = pg_pool.tile([GS, GS * K], f32, tag="bigD")
                for r in range(NH):
                    nc.tensor.matmul(
                        bigD[:],
                        featsT[:, r * BL + g * GS:r * BL + (g + 1) * GS],
                        wd_sb[:, (g * NH + r) * GS * K:(g * NH + r + 1) * GS * K],
                        start=(r == 0), stop=(r == NH - 1),
                    )
                zm = work.tile([GS, GS * K], f32, tag="zm")
                nc.vector.tensor_mul(zm[:], bigD[:], m64_sb[:])
                zg = work.tile([GS, K], f32, tag="zg")
                nc.vector.tensor_reduce(
                    zg[:], zm[:].rearrange("p (g j) -> p j g", j=K),
                    _mb.AxisListType.X, AT.add)
                zb = work.tile([GS, K], f32, tag="zb")
                nc.vector.tensor_add(zb[:], zg[:], bd8_sb[:, g * K:(g + 1) * K])
                h1g = work.tile([GS, K], f32, tag="h1g")
                nc.scalar.activation(
                    h1g[:], zb[:], getattr(_mb.ActivationFunctionType, act_fn))
                pth = pt_pool.tile([K, GS], f32, tag="pt")
                nc.tensor.transpose(pth[:], h1g[:], id_sb[:])
                nc.vector.tensor_copy(h1T[:, g * GS:(g + 1) * GS], pth[:])
                # heads: feats part + adapter part (diagonal-extracted)
                pB = pg_pool.tile([GS, 5], f32, tag="pB")
                for r in range(NH):
                    nc.tensor.matmul(
                        pB[:],
                        featsT[:, r * BL + g * GS:r * BL + (g + 1) * GS],
                        wro_sb[:, r * 5:(r + 1) * 5],
                        start=(r == 0), stop=(r == NH - 1),
                    )
                pA = pg_pool.tile([GS, GS * 5], f32, tag="pA")
                nc.tensor.matmul(
                    pA[:], h1T[:, g * GS:(g + 1) * GS],
                    wuro_sb[:, g * GS * 5:(g + 1) * GS * 5],
                    start=True, stop=True)
                am = work.tile([GS, GS * 5], f32, tag="am")
                nc.vector.tensor_mul(am[:], pA[:], m5_sb[:])
                rg = work.tile([GS, 5], f32, tag="rg")
                nc.vector.tensor_reduce(
                    rg[:], am[:].rearrange("p (g j) -> p j g", j=5),
                    _mb.AxisListType.X, AT.add)
                o1 = work.tile([GS, 5], f32, tag="o1")
                nc.vector.tensor_add(o1[:], pB[:], rg[:])
                og = work.tile([GS, 5], f32, tag="og")
                nc.vector.tensor_add(og[:], o1[:], bro8_sb[:, g * 5:(g + 1) * 5])
                nc.sync.dma_start(out.ap()[g * GS:(g + 1) * GS, :], og[:])

    nc.compile()
    return nc


def _softmax(x):
    e = np.exp(x - x.max())
    return e / e.sum()


def _prepare_inputs(hidden_states, attention_mask, section_id, layer_weights,
                    W_down, b_down, W_up, b_up, W_reg, b_reg, W_ord, b_ord):
    hidden_states = np.asarray(hidden_states, dtype=np.float32)
    mask = np.asarray(attention_mask)
    sec = np.asarray(section_id).astype(np.int64)
    lw = np.asarray(layer_weights, dtype=np.float32)
    W_down = np.asarray(W_down, dtype=np.float32)
    b_down = np.asarray(b_down, dtype=np.float32)
    W_up = np.asarray(W_up, dtype=np.float32)
    b_up = np.asarray(b_up, dtype=np.float32)
    W_reg = np.asarray(W_reg, dtype=np.float32)
    b_reg = np.asarray(b_reg, dtype=np.float32)
    W_ord = np.asarray(W_ord, dtype=np.float32)
    b_ord = np.asarray(b_ord, dtype=np.float32)

    w = _softmax(lw)
    mf = mask.astype(np.float32)
    msum = np.maximum(mf.sum(axis=1), 1e-6)
    cmask = mf / msum[:, None]
    base = cmask.reshape(B, 2, 128).transpose(2, 1, 0)          # [p, c, b]
    coeff_all = w[None, :, None, None] * base[:, None, :, :]    # [p, l, c, b]

    wro_np = np.concatenate([W_reg, W_ord], axis=1)             # [H, 5]
    wro_dev = np.ascontiguousarray(wro_np.reshape(128, NH * 5))
    wd_all = W_down[sec]                                        # [B, H, K]
    wd_pbrk = wd_all.reshape(B, 128, NH, K).transpose(1, 0, 2, 3)  # [p, b, r, k]
    wu_ro = np.einsum("ekh,hj->ekj", W_up, wro_np)              # [NSEC, K, 5]
    wuro_all = wu_ro[sec].transpose(1, 0, 2)                    # [K, B, 5]
    bd_all = b_down[sec]                                        # [B, K]
    bro_all = b_up[sec] @ wro_np + np.concatenate([b_reg, b_ord])[None]

    m64_np = np.zeros((GS, GS * K), np.float32)
    m5_np = np.zeros((GS, GS * 5), np.float32)
    for i in range(GS):
        m64_np[i, i * K:(i + 1) * K] = 1.0
        m5_np[i, i * 5:(i + 1) * 5] = 1.0
    ones_np = np.zeros((128, GS * GS), np.float32)
    for j in range(GS):
        ones_np[:, j * GS + j] = 1.0
    ident = np.eye(GS, dtype=np.float32)

    in_maps = []
    for core in range(NCORES):
        sl = slice(core * BL, (core + 1) * BL)
        wd_core = wd_pbrk[:, sl]                                # [p, BL, r, k]
        wd2_np = wd_core.reshape(128, NG, GS, NH, K).transpose(0, 1, 3, 2, 4)
        in_maps.append({
            "hs": np.ascontiguousarray(hidden_states[:, sl]),
            "cs": np.ascontiguousarray(
                coeff_all[:, :, :, sl].reshape(128, L * 2 * BL)),
            "ones8": ones_np,
            "wd2": np.ascontiguousarray(wd2_np.reshape(128, NG * NH * GS * K)),
            "wro": wro_dev,
            "wuro": np.ascontiguousarray(wuro_all[:, sl].reshape(K, BL * 5)),
            "m64": m64_np,
            "m5": m5_np,
            "bd8": np.ascontiguousarray(
                bd_all[sl].reshape(NG, GS, K).transpose(1, 0, 2)
                .reshape(GS, NG * K)),
            "bro8": np.ascontiguousarray(
                bro_all[sl].reshape(NG, GS, 5).transpose(1, 0, 2)
                .reshape(GS, NG * 5)),
            "ident": ident,
        })
    return in_maps


def get_module(act_fn="Gelu"):
    key = "nc_" + act_fn
    if key not in _CACHE:
        _CACHE[key] = _build_module(act_fn)
    return _CACHE[key]


def kernel(hidden_states, attention_mask, section_id, layer_weights,
           W_down, b_down, W_up, b_up, W_reg, b_reg, W_ord, b_ord):
    global LAST_RESULT
    from concourse.bass_utils import run_bass_kernel_spmd

    in_maps = _prepare_inputs(
        hidden_states, attention_mask, section_id, layer_weights,
        W_down, b_down, W_up, b_up, W_reg, b_reg, W_ord, b_ord)
    nc = get_module()
    res = run_bass_kernel_spmd(nc, in_maps, list(range(NCORES)))
    LAST_RESULT = res
    out = np.concatenate([res.results[c]["out"] for c in range(NCORES)], axis=0)
    reg = np.ascontiguousarray(out[:, 0])
    ord_logits = np.ascontiguousarray(out[:, 1:5])
    return reg, ord_logits
